# revision 32
# baseline (speedup 1.0000x reference)
"""Trainium2 Bass kernel for the Tacotron-style attention decoder.

Sharding: data-parallel over batch. B=128 split across 8 cores (16 per
core); all weights replicated; the sequential scan over T_dec=256
decoder steps runs locally per core inside one Bass program.

Layout strategy ("T-layout"): activations live feature-on-partition,
batch-on-free ([feat<=128, 16]).  Linear layers run weight-stationary
(lhsT = W.T chunk [128_in, <=128_out], rhs = x.T [128_in, 16]) so
outputs land in PSUM already in T-layout and per-feature biases fuse
into ACT/DVE ops.  The attention tanh runs per-batch with h on the
partition axis so `keys + q` is a per-partition tensor_scalar add and
the v-dot / context matmuls are M=1 col-packed (tile_position) PE ops.
Sigmoid is computed as 0.5*tanh(x/2)+0.5 so the whole kernel uses one
ACT table set (exp_and_others: Tanh/Exp/Relu).
"""

import numpy as np
from contextlib import ExitStack

import concourse.bacc as bacc
import concourse.bass as bass
import concourse.mybir as mybir
import concourse.tile as tile
from concourse.bass_utils import run_bass_kernel_spmd
from concourse.masks import make_identity

F32 = mybir.dt.float32
BF16 = mybir.dt.bfloat16
AF = mybir.ActivationFunctionType
ALU = mybir.AluOpType

NCORES = 8
B = 16          # batch per core
TDEC = 256
TENC = 512
H = 256
INR = 400
INR_PAD = 512

# dtype of the keys / (keys+q) / tanh attention path (bf16 keeps SBUF
# under the 192KB/partition tile budget; fp32 does not fit)
KDT = BF16


# ---------------------------------------------------------------- program

def _dram_in(nc, name, shape, dt=F32):
    return nc.dram_tensor(name, list(shape), dt, kind="ExternalInput")


def _gru_matmuls(nc, wih_sb, whh_sb, n_in_ch, x_chunks, h_sb, ps_rz, ps_n):
    """Emit the 6*(n_in_ch+2) weight-stationary matmuls of one GRU cell.

    ps_rz accumulates ih+hh for the r,z gates (4 chunks of 128 feats).
    ps_n holds the n-part: [:, 0:2] the ih half, [:, 2:4] the hh half
    (kept separate because of the r * (hn + bhn) structure).
    """
    # mc outer: one PSUM bank can only hold one open accumulation group,
    # so each 128-feature slice's group must fully close before the next
    # opens in the same bank.
    for mc in range(4):
        for kc in range(n_in_ch):
            nc.tensor.matmul(ps_rz[:, mc, :],
                             wih_sb[:, kc, mc * 128:(mc + 1) * 128],
                             x_chunks[kc], start=(kc == 0), stop=False)
        for kc in range(2):
            nc.tensor.matmul(ps_rz[:, mc, :],
                             whh_sb[:, kc, mc * 128:(mc + 1) * 128],
                             h_sb[:, kc, :], start=False, stop=(kc == 1))
    for mc in range(4, 6):
        for kc in range(n_in_ch):
            nc.tensor.matmul(ps_n[:, mc - 4, :],
                             wih_sb[:, kc, mc * 128:(mc + 1) * 128],
                             x_chunks[kc], start=(kc == 0),
                             stop=(kc == n_in_ch - 1))
        for kc in range(2):
            nc.tensor.matmul(ps_n[:, mc - 2, :],
                             whh_sb[:, kc, mc * 128:(mc + 1) * 128],
                             h_sb[:, kc, :], start=(kc == 0), stop=(kc == 1))


def _gru_ew(nc, sb, ps_rz, ps_n, brz_rep, bin_rep, bhn05_rep, h_sb):
    """GRU gate elementwise chain, sigmoid via tanh.  Updates h_sb in place."""
    grz = sb.tile([128, 4, B], F32, tag="grz")
    nc.vector.tensor_add(grz, ps_rz, brz_rep)
    trz = sb.tile([128, 4, B], F32, tag="trz")
    # trz = tanh(0.5*(g+b));  sigmoid(x) = 0.5*trz+0.5
    nc.scalar.activation(trz, grz, AF.Tanh, scale=0.5)
    # hnb05 = 0.5*(gh_n + bhn) = 0.5*gh_n + (0.5*bhn)
    hnb05 = sb.tile([128, 2, B], F32, tag="hnb")
    nc.vector.scalar_tensor_tensor(hnb05, ps_n[:, 2:4, :], 0.5, bhn05_rep,
                                   op0=ALU.mult, op1=ALU.add)
    inb = sb.tile([128, 2, B], F32, tag="inb")
    nc.vector.tensor_add(inb, ps_n[:, 0:2, :], bin_rep)
    # rhn = (trz_r + 1) * hnb05 = (2r)*(0.5 hnb) = r*hnb
    rhn = sb.tile([128, 2, B], F32, tag="rhn")
    nc.vector.scalar_tensor_tensor(rhn, trz[:, 0:2, :], 1.0, hnb05,
                                   op0=ALU.add, op1=ALU.mult)
    na = sb.tile([128, 2, B], F32, tag="na")
    nc.vector.tensor_add(na, inb, rhn)
    n = sb.tile([128, 2, B], F32, tag="ngate")
    nc.scalar.activation(n, na, AF.Tanh)
    hmn = sb.tile([128, 2, B], F32, tag="hmn")
    nc.vector.tensor_sub(hmn, h_sb, n)
    # zh2 = (trz_z + 1)*(h-n) = 2z*(h-n)
    zh2 = sb.tile([128, 2, B], F32, tag="zh2")
    nc.vector.scalar_tensor_tensor(zh2, trz[:, 2:4, :], 1.0, hmn,
                                   op0=ALU.add, op1=ALU.mult)
    # h' = n + 0.5*zh2
    nc.vector.scalar_tensor_tensor(h_sb, zh2, 0.5, n,
                                   op0=ALU.mult, op1=ALU.add)


def build_program(tdec=TDEC):
    nc = bacc.Bacc()

    # ---- dram I/O
    d_xsT = _dram_in(nc, "xsT", [INR_PAD, tdec, B])
    d_encd = _dram_in(nc, "enc_d", [B, 2, 128, TENC])
    d_w1T = _dram_in(nc, "w1T", [INR_PAD, 256])
    d_w2T = _dram_in(nc, "w2T", [256, 128])
    d_wihA = _dram_in(nc, "wihT_a", [384, 768])
    d_whhA = _dram_in(nc, "whhT_a", [256, 768])
    d_wqT = _dram_in(nc, "wqT", [256, 256])
    d_wkT = _dram_in(nc, "wkT", [256, 256])
    d_wLT = _dram_in(nc, "woutLT", [256, 256])
    d_wRT = _dram_in(nc, "woutRT", [256, 256])
    d_d1ih = _dram_in(nc, "d1ihT", [256, 768])
    d_d1hh = _dram_in(nc, "d1hhT", [256, 768])
    d_d2ih = _dram_in(nc, "d2ihT", [256, 768])
    d_d2hh = _dram_in(nc, "d2hhT", [256, 768])
    d_projT = _dram_in(nc, "projT", [256, INR_PAD])
    d_v2 = _dram_in(nc, "v2", [128, 2])
    d_b1r = _dram_in(nc, "b1r", [128, 2])
    d_b2r = _dram_in(nc, "b2r", [128, 1])
    d_bqr = _dram_in(nc, "bqr", [128, 2])
    d_boutr = _dram_in(nc, "boutr", [128, 2])
    d_projbr = _dram_in(nc, "projbr", [128, 4])
    d_reps = {}
    for g in ("a", "d1", "d2"):
        d_reps[g + "_brz"] = _dram_in(nc, g + "_brz", [128, 4, B])
        d_reps[g + "_bin"] = _dram_in(nc, g + "_bin", [128, 2, B])
        d_reps[g + "_bhn05"] = _dram_in(nc, g + "_bhn05", [128, 2, B])

    d_outsT = nc.dram_tensor("outsT", [tdec, 128, 4, B], F32,
                             kind="ExternalOutput")
    d_aligns = nc.dram_tensor("aligns", [tdec, B, TENC], F32,
                              kind="ExternalOutput")

    with TileCtx(nc) as tc, ExitStack() as big:
        wpool = big.enter_context(tc.tile_pool(name="wpool", bufs=1))

        def _load(dram, shape, tag, dt=F32):
            t = wpool.tile(list(shape), dt, tag=tag)
            nc.sync.dma_start(out=t, in_=dram[:])
            return t

        # weights as [128, kchunks, out]
        def _loadw(dram, in_dim, out_dim, tag):
            kch = in_dim // 128
            t = wpool.tile([128, kch, out_dim], F32, tag=tag)
            for kc in range(kch):
                nc.sync.dma_start(out=t[:, kc, :],
                                  in_=dram[kc * 128:(kc + 1) * 128, :])
            return t

        w1T = _loadw(d_w1T, INR_PAD, 256, "w1T")
        w2T = _loadw(d_w2T, 256, 128, "w2T")
        wihA = _loadw(d_wihA, 384, 768, "wihA")
        whhA = _loadw(d_whhA, 256, 768, "whhA")
        wqT = _loadw(d_wqT, 256, 256, "wqT")
        wkT = _loadw(d_wkT, 256, 256, "wkT")
        wLT = _loadw(d_wLT, 256, 256, "wLT")
        wRT = _loadw(d_wRT, 256, 256, "wRT")
        d1ih = _loadw(d_d1ih, 256, 768, "d1ih")
        d1hh = _loadw(d_d1hh, 256, 768, "d1hh")
        d2ih = _loadw(d_d2ih, 256, 768, "d2ih")
        d2hh = _loadw(d_d2hh, 256, 768, "d2hh")
        projT = _loadw(d_projT, 256, INR_PAD, "projT")

        v2f = _load(d_v2, [128, 2], "v2f")
        # v replicated over 32 columns: the scores matmul uses M=32 with 32
        # identical stationary columns so all PSUM partitions get written
        # (engines cannot address sub-32 partition groups).
        v2r = wpool.tile([128, 2, 32], KDT, tag="v2r")
        for hc in range(2):
            nc.vector.tensor_copy(v2r[:, hc, :],
                                  v2f[:, hc:hc + 1].to_broadcast([128, 32]))
        b1r = _load(d_b1r, [128, 2], "b1r")
        b2r = _load(d_b2r, [128, 1], "b2r")
        bqr = _load(d_bqr, [128, 2], "bqr")
        boutr = _load(d_boutr, [128, 2], "boutr")
        projbr = _load(d_projbr, [128, 4], "projbr")
        reps = {k: _load(d, [128, 4 if k.endswith("brz") else 2, B], k)
                for k, d in d_reps.items()}

        ident = wpool.tile([128, 128], F32, tag="ident")
        make_identity(nc, ident)

        # persistent state / precomputed activations
        p2T = wpool.tile([128, tdec, B], F32, tag="p2T")
        h_aT = wpool.tile([128, 2, B], F32, tag="h_aT")
        h1T = wpool.tile([128, 2, B], F32, tag="h1T")
        h2T = wpool.tile([128, 2, B], F32, tag="h2T")
        aoT = wpool.tile([128, 2, B], F32, tag="aoT")
        for t in (h_aT, h1T, h2T, aoT):
            nc.vector.memset(t, 0.0)

        # ---------------- precompute 1: prenet
        with ExitStack() as pre:
            pps = pre.enter_context(
                tc.tile_pool(name="pps", bufs=4, space="PSUM"))
            pin = pre.enter_context(tc.tile_pool(name="pin", bufs=3))

            ncols = tdec * B            # 4096 flat (t, b) columns
            csz = min(512, ncols)
            nchunks = ncols // csz
            p1T = pre.enter_context(tc.tile_pool(name="p1pool", bufs=1)) \
                .tile([128, 2, ncols], F32, tag="p1T")
            for nch in range(nchunks):
                xin = pin.tile([128, 4, csz], F32, tag="xin")
                for kc in range(4):
                    nc.sync.dma_start(
                        out=xin[:, kc, :],
                        in_=d_xsT[kc * 128:(kc + 1) * 128]
                        .rearrange("p t b -> p (t b)")
                        [:, nch * csz:(nch + 1) * csz])
                for mc in range(2):
                    ps = pps.tile([128, csz], F32, tag="ps")
                    for kc in range(4):
                        nc.tensor.matmul(ps, w1T[:, kc, mc * 128:(mc + 1) * 128],
                                         xin[:, kc, :],
                                         start=(kc == 0), stop=(kc == 3))
                    nc.scalar.activation(p1T[:, mc, nch * csz:(nch + 1) * csz],
                                         ps, AF.Relu, bias=b1r[:, mc:mc + 1])
            for nch in range(nchunks):
                ps = pps.tile([128, csz], F32, tag="ps")
                for kc in range(2):
                    nc.tensor.matmul(ps, w2T[:, kc, :],
                                     p1T[:, kc, nch * csz:(nch + 1) * csz],
                                     start=(kc == 0), stop=(kc == 1))
                nc.scalar.activation(
                    p2T.rearrange("p t b -> p (t b)")[:, nch * csz:(nch + 1) * csz],
                    ps, AF.Relu, bias=b2r[:, 0:1])

        # ---------------- precompute 2: keys + encW per batch element
        # (separate pool so the prenet transients' SBUF is reusable)
        kpool = big.enter_context(tc.tile_pool(name="kpool", bufs=1))
        keys_sb = kpool.tile([128, 2, B, TENC], KDT, tag="keys")
        encW_sb = kpool.tile([128, B, 4, H], KDT, tag="encW")
        with ExitStack() as pre2:
            pps = pre2.enter_context(
                tc.tile_pool(name="pps2", bufs=4, space="PSUM"))
            pin = pre2.enter_context(tc.tile_pool(name="pin2", bufs=3))
            for b in range(B):
                ed = pin.tile([128, 2, TENC], F32, tag="encd")
                for dc in range(2):
                    nc.sync.dma_start(out=ed[:, dc, :], in_=d_encd[b, dc])
                for hc in range(2):
                    ps = pps.tile([128, 512], F32, tag="ps")
                    for dc in range(2):
                        nc.tensor.matmul(ps, wkT[:, dc, hc * 128:(hc + 1) * 128],
                                         ed[:, dc, :],
                                         start=(dc == 0), stop=(dc == 1))
                    nc.vector.tensor_copy(keys_sb[:, hc, b, :], ps)
                for tc4 in range(4):
                    pse = pps.tile([128, H], F32, tag="pse")
                    for dc in range(2):
                        nc.tensor.matmul(
                            pse, ed[:, dc, tc4 * 128:(tc4 + 1) * 128],
                            wLT[:, dc, :], start=(dc == 0), stop=(dc == 1))
                    nc.vector.tensor_copy(encW_sb[:, b, tc4, :], pse)

        # ---------------- main decoder loop
        ps_small = big.enter_context(
            tc.tile_pool(name="ps_small", bufs=1, space="PSUM"))
        ps_sc = big.enter_context(
            tc.tile_pool(name="ps_sc", bufs=2, space="PSUM"))
        ps_at = big.enter_context(
            tc.tile_pool(name="ps_at", bufs=2, space="PSUM"))
        ps_tr = big.enter_context(
            tc.tile_pool(name="ps_tr", bufs=2, space="PSUM"))
        sb = big.enter_context(tc.tile_pool(name="sb", bufs=2))
        qpool = big.enter_context(tc.tile_pool(name="qpool", bufs=3))

        def step(i):
            # ---- attention GRU
            ps_rz = ps_small.tile([128, 4, B], F32, tag="psg")
            ps_n = ps_small.tile([128, 4, B], F32, tag="psg2")
            x_chunks = [p2T[:, i, :], aoT[:, 0, :], aoT[:, 1, :]]
            _gru_matmuls(nc, wihA, whhA, 3, x_chunks, h_aT, ps_rz, ps_n)
            _gru_ew(nc, sb, ps_rz, ps_n,
                    reps["a_brz"], reps["a_bin"], reps["a_bhn05"], h_aT)

            # ---- q = Wq h_a + bq ;  aoR = WoutR h_a + bout
            ps_q = ps_small.tile([128, 2, B], F32, tag="psg")
            ps_r = ps_small.tile([128, 2, B], F32, tag="psg2")
            for mc in range(2):
                for kc in range(2):
                    nc.tensor.matmul(ps_q[:, mc, :],
                                     wqT[:, kc, mc * 128:(mc + 1) * 128],
                                     h_aT[:, kc, :],
                                     start=(kc == 0), stop=(kc == 1))
                    nc.tensor.matmul(ps_r[:, mc, :],
                                     wRT[:, kc, mc * 128:(mc + 1) * 128],
                                     h_aT[:, kc, :],
                                     start=(kc == 0), stop=(kc == 1))
            q_sb = sb.tile([128, 2, B], F32, tag="q_sb")
            aoR = sb.tile([128, 2, B], F32, tag="aoR")
            for mc in range(2):
                nc.vector.tensor_scalar_add(q_sb[:, mc, :], ps_q[:, mc, :],
                                            bqr[:, mc:mc + 1])
                nc.vector.tensor_scalar_add(aoR[:, mc, :], ps_r[:, mc, :],
                                            boutr[:, mc:mc + 1])

            # ---- attention: scores -> softmax -> context, one group of 4
            # batch elems at a time.  Each b's score row lives at psum
            # partition 32j (tile_position col-packing); softmax runs on the
            # full [128, TENC] tile (garbage rows are harmless: scores are
            # bounded so exp without max-subtraction is safe), and the
            # align/ctx rows are extracted via PE transpose + free-dim
            # strided gather (engines cannot move data across partitions).
            alignT = sb.tile([128, 4, B], KDT, tag="alignT")
            for g in range(4):
                qs = qpool.tile([128, 2, 4, TENC], KDT, tag="qs")
                for hc in range(2):
                    for j in range(4):
                        b = g * 4 + j
                        nc.vector.tensor_scalar_add(
                            qs[:, hc, j, :], keys_sb[:, hc, b, :],
                            q_sb[:, hc, b:b + 1])
                nc.scalar.activation(qs, qs, AF.Tanh)
                ps_s = ps_sc.tile([128, TENC], F32, tag="ps_s")
                for j in range(4):
                    for hc in range(2):
                        nc.tensor.matmul(
                            ps_s[32 * j:32 * (j + 1), :], v2r[:, hc, :],
                            qs[:, hc, j, :], start=(hc == 0), stop=(hc == 1),
                            tile_position=(0, 32 * j))
                expv = sb.tile([128, TENC], F32, tag="expv")
                den = sb.tile([128, 1], F32, tag="den")
                nc.scalar.activation(expv, ps_s, AF.Exp, accum_out=den)
                rden = sb.tile([128, 1], F32, tag="rden")
                nc.vector.reciprocal(rden, den)
                align = sb.tile([128, TENC], F32, tag="align")
                nc.vector.tensor_scalar_mul(align, expv, rden)
                nc.sync.dma_start(out=d_aligns[i, g * 4:(g + 1) * 4, :],
                                  in_=align[0:128:32, :])
                for tc4 in range(4):
                    pst = ps_tr.tile([128, 128], F32, tag="pst")
                    nc.tensor.transpose(
                        pst, align[:, tc4 * 128:(tc4 + 1) * 128], ident)
                    nc.vector.tensor_copy(alignT[:, tc4, g * 4:(g + 1) * 4],
                                          pst[:, 0:128:32])

            # ---- attn_out (ctx @ WoutL.T folded via encW)
            aoLT = sb.tile([128, 2, B], F32, tag="aoLT")
            for g in range(4):
                ps_a = ps_at.tile([128, H], F32, tag="ps_a")
                for j in range(4):
                    b = g * 4 + j
                    for tc4 in range(4):
                        nc.tensor.matmul(
                            ps_a[32 * j:32 * (j + 1), :],
                            alignT[:, tc4, b:b + 1].to_broadcast([128, 32]),
                            encW_sb[:, b, tc4, :],
                            start=(tc4 == 0), stop=(tc4 == 3),
                            tile_position=(0, 32 * j))
                a32 = sb.tile([128, H], F32, tag="a32")
                nc.vector.tensor_copy(a32, ps_a)
                for mc in range(2):
                    pst = ps_tr.tile([128, 128], F32, tag="pst")
                    nc.tensor.transpose(
                        pst, a32[:, mc * 128:(mc + 1) * 128], ident)
                    nc.vector.tensor_copy(aoLT[:, mc, g * 4:(g + 1) * 4],
                                          pst[:, 0:128:32])
            # aoT = aoL + aoR  (write the carry)
            nc.vector.tensor_add(aoT, aoLT, aoR)

            # ---- decoder GRU 1
            ps_rz = ps_small.tile([128, 4, B], F32, tag="psg")
            ps_n = ps_small.tile([128, 4, B], F32, tag="psg2")
            _gru_matmuls(nc, d1ih, d1hh, 2, [aoT[:, 0, :], aoT[:, 1, :]],
                         h1T, ps_rz, ps_n)
            _gru_ew(nc, sb, ps_rz, ps_n,
                    reps["d1_brz"], reps["d1_bin"], reps["d1_bhn05"], h1T)

            dinT = sb.tile([128, 2, B], F32, tag="dinT")
            nc.vector.tensor_add(dinT, h1T, aoT)

            # ---- decoder GRU 2
            ps_rz = ps_small.tile([128, 4, B], F32, tag="psg")
            ps_n = ps_small.tile([128, 4, B], F32, tag="psg2")
            _gru_matmuls(nc, d2ih, d2hh, 2, [dinT[:, 0, :], dinT[:, 1, :]],
                         h2T, ps_rz, ps_n)
            _gru_ew(nc, sb, ps_rz, ps_n,
                    reps["d2_brz"], reps["d2_bin"], reps["d2_bhn05"], h2T)

            # ---- projection to mel (out dim padded 400 -> 512)
            ps_p = ps_small.tile([128, 4, B], F32, tag="psg")
            for mi in range(4):
                for kc in range(2):
                    nc.tensor.matmul(ps_p[:, mi, :],
                                     projT[:, kc, mi * 128:(mi + 1) * 128],
                                     h2T[:, kc, :],
                                     start=(kc == 0), stop=(kc == 1))
            out_sb = sb.tile([128, 4, B], F32, tag="out_sb")
            for mi in range(4):
                nc.vector.tensor_scalar_add(out_sb[:, mi, :],
                                            ps_p[:, mi, :],
                                            projbr[:, mi:mi + 1])
            nc.sync.dma_start(out=d_outsT[i], in_=out_sb)

        with tc.For_i(0, tdec, 1) as i:
            step(i)

    nc.compile()
    return nc


def TileCtx(nc):
    return tile.TileContext(nc)


# ---------------------------------------------------------------- host glue

def _prep_shared(inputs):
    """Weight/bias arrays shared by all cores (host-side relayout only)."""
    f32 = np.float32
    sh = {}
    w1T = np.zeros((INR_PAD, 256), f32)
    w1T[:INR] = inputs["prenet_W1"].T
    sh["w1T"] = w1T
    sh["w2T"] = np.ascontiguousarray(inputs["prenet_W2"].T, f32)
    sh["wihT_a"] = np.ascontiguousarray(inputs["arnn_Wih"].T, f32)
    sh["whhT_a"] = np.ascontiguousarray(inputs["arnn_Whh"].T, f32)
    sh["wqT"] = np.ascontiguousarray(inputs["attn_Wq"].T, f32)
    sh["wkT"] = np.ascontiguousarray(inputs["attn_Wk"].T, f32)
    wout = inputs["attn_Wout"]
    sh["woutLT"] = np.ascontiguousarray(wout[:, :H].T, f32)
    sh["woutRT"] = np.ascontiguousarray(wout[:, H:].T, f32)
    sh["d1ihT"] = np.ascontiguousarray(inputs["dec1_Wih"].T, f32)
    sh["d1hhT"] = np.ascontiguousarray(inputs["dec1_Whh"].T, f32)
    sh["d2ihT"] = np.ascontiguousarray(inputs["dec2_Wih"].T, f32)
    sh["d2hhT"] = np.ascontiguousarray(inputs["dec2_Whh"].T, f32)
    projT = np.zeros((256, INR_PAD), f32)
    projT[:, :INR] = inputs["proj_W"].T
    sh["projT"] = projT
    sh["v2"] = np.ascontiguousarray(inputs["attn_v"].reshape(2, 128).T, f32)

    def chunked(b, nch):
        return np.ascontiguousarray(b.reshape(nch, 128).T, f32)

    sh["b1r"] = chunked(inputs["prenet_b1"], 2)
    sh["b2r"] = chunked(inputs["prenet_b2"], 1)
    sh["bqr"] = chunked(inputs["attn_bq"], 2)
    sh["boutr"] = chunked(inputs["attn_bout"], 2)
    pb = np.zeros(INR_PAD, f32)
    pb[:INR] = inputs["proj_b"]
    sh["projbr"] = chunked(pb, 4)

    def rep(vec, nch):
        return np.ascontiguousarray(
            np.broadcast_to(vec.reshape(nch, 128).T[:, :, None],
                            (128, nch, B)), f32)

    for g, ih, hh in (("a", "arnn_bih", "arnn_bhh"),
                      ("d1", "dec1_bih", "dec1_bhh"),
                      ("d2", "dec2_bih", "dec2_bhh")):
        bih, bhh = np.asarray(inputs[ih]), np.asarray(inputs[hh])
        sh[g + "_brz"] = rep(bih[:2 * H] + bhh[:2 * H], 4)
        sh[g + "_bin"] = rep(bih[2 * H:], 2)
        sh[g + "_bhn05"] = rep(0.5 * bhh[2 * H:], 2)
    return sh


def _prep_core(inputs, c, tdec):
    f32 = np.float32
    s = slice(c * B, (c + 1) * B)
    enc = np.asarray(inputs["encoder_outputs"][s], f32)   # [B, TENC, H]
    inp = np.asarray(inputs["inputs"][s], f32)            # [B, tdec, INR]
    xs = np.zeros((B, tdec, INR), f32)
    xs[:, 1:, :] = inp[:, :tdec - 1, :]
    xsT = np.zeros((INR_PAD, tdec, B), f32)
    xsT[:INR] = xs.transpose(2, 1, 0)
    enc_d = np.ascontiguousarray(
        enc.transpose(0, 2, 1).reshape(B, 2, 128, TENC), f32)
    return {"xsT": xsT, "enc_d": enc_d}


_cached = {}


def _get_program(tdec):
    if tdec not in _cached:
        _cached[tdec] = build_program(tdec)
    return _cached[tdec]


def kernel(**inputs):
    return run_sharded(inputs, TDEC)


def run_sharded(inputs, tdec, trace=False):
    nc = _get_program(tdec)
    sh = _prep_shared(inputs)
    in_maps = []
    for c in range(NCORES):
        m = dict(sh)
        m.update(_prep_core(inputs, c, tdec))
        in_maps.append(m)
    res = run_bass_kernel_spmd(nc, in_maps, list(range(NCORES)), trace=trace)
    outs = []
    aligns = []
    for c in range(NCORES):
        oT = res.results[c]["outsT"]          # [tdec, 128, 4, B]
        al = res.results[c]["aligns"]         # [tdec, B, TENC]
        outs.append(oT.transpose(3, 0, 2, 1).reshape(B, tdec, 512)[:, :, :INR])
        aligns.append(al.transpose(1, 0, 2))
    outputs = np.concatenate(outs, axis=0)
    alignments = np.concatenate(aligns, axis=0)
    if trace:
        return (outputs, alignments), res
    return outputs, alignments
